# revision 1
# baseline (speedup 1.0000x reference)
"""MoE (noisy top-2 routing, 8 experts) on 8 Trainium2 NeuronCores.

Strategy (expert-parallel, per the sharding hint):
  - Router runs on host in float64 (134 MFLOP — negligible next to the
    137 GFLOP expert MLPs; fp64 makes the top-k selection robust).
  - Tokens are dispatched by top-2 expert id; core e gets expert e's
    weights and its routed tokens (padded to a fixed capacity).
  - Each core computes o = relu(x @ W1[e] + b1[e]) @ W2[e] for its
    tokens with fp16 matmuls (full PE rate, fp32 PSUM accumulation;
    ~4e-4 scale-relative output error).
  - Host combines: out[t] = g1*(o_e1[t] + b2[e1]) + g2*(o_e2[t] + b2[e2]).

Layout notes: activations stay feature-major on device (xT [D, C],
hT [F, C]) so both matmul layers contract along the partition dim with
zero on-device transposes. xT, W2, b1 and the intermediate hT are all
SBUF-resident (fp16): layer 1 streams only W1 tiles from DRAM and its
PSUM-evict reducer writes relu(psum+b1) straight into hT slices;
layer 2 reads hT/W2 slices with no DMA at all and streams o out.
"""

import numpy as np

import concourse.bass as bass
import concourse.mybir as mybir
import concourse.tile as tile
from concourse import bacc
from concourse.bass import ts
from concourse.bass_utils import run_bass_kernel_spmd
from concourse.kernels.tile_matmul import (
    ShapeInfo,
    dma_from_dram_kxm,
    composable_matmul_tile_kernel,
)

B, S, D, F, E = 2, 2048, 1024, 4096, 8
T = B * S
TOP_K = 2
N_CORES = 8
CHUNK = 384   # capacity granularity: multiples of 384 keep layer 2's
              # token dim on 128-partition boundaries with M_TILE=384
CAP = 1152    # tokens per expert, padded; key(0) inputs max out at 1064

_build_cache = {}
_last_run = None


def profile_last(trace_cores=None):
    """Re-run the most recent kernel() dispatch with NTFF tracing; returns
    BassKernelResults (exec_time_ns etc.). Dev-harness helper only."""
    nc, in_maps = _last_run
    return run_bass_kernel_spmd(nc, in_maps, list(range(N_CORES)),
                                trace=True, trace_cores=trace_cores)


def _l1_segments(csum):
    """Split the computed token count into near-equal layer-1 moving-dim
    batches <=512 wide (PSUM bank limit). Equal-ish widths keep every
    batch >=257 so the fp16 ldweights stays hidden under the stream."""
    nseg = -(-csum // 512)
    seg = -(-csum // nseg)
    segs = [seg] * (nseg - 1)
    segs.append(csum - seg * (nseg - 1))
    return tuple(segs)


def _build(cap, csum):
    """Compile the per-core expert-MLP kernel (SPMD: same program, per-core
    weights/tokens). cap is the xT/o array width (384-multiple); csum
    (<= cap) is how many token columns layer 1 actually computes."""
    if (cap, csum) in _build_cache:
        return _build_cache[(cap, csum)]

    f32 = mybir.dt.float32
    f16 = mybir.dt.float16

    nc = bacc.Bacc("TRN2", target_bir_lowering=False, debug=False,
                   num_devices=N_CORES)
    xT = nc.dram_tensor("xT", [D, cap], f16, kind="ExternalInput")
    w1 = nc.dram_tensor("w1", [D, F], f16, kind="ExternalInput")
    b1 = nc.dram_tensor("b1", [128, F // 128], f32, kind="ExternalInput")
    w2 = nc.dram_tensor("w2", [F, D], f16, kind="ExternalInput")
    oT = nc.dram_tensor("oT", [D, cap], f32, kind="ExternalOutput")

    with tile.TileContext(nc) as tc:
        from contextlib import ExitStack
        with ExitStack() as octx:
            resident = octx.enter_context(tc.tile_pool(name="resident", bufs=1))
            hT = resident.tile([128, F // 128, cap], f16)
            xT_sb = resident.tile([128, D // 128, cap], f16)
            b1_tile = resident.tile([128, F // 128], f32)
            nc.sync.dma_start(b1_tile[:], b1.ap())
            # xT rides the scalar HWDGE ring (empty at startup; W1 owns the
            # sync ring) chunked by token block so the first PSUM group's
            # chunk lands first
            segs = _l1_segments(csum)
            seg_starts = [sum(segs[:i]) for i in range(len(segs))]
            xT_tiled = xT.ap().rearrange("(po pi) f -> pi po f", pi=128)
            for st, ln in zip(seg_starts, segs):
                nc.scalar.dma_start(xT_sb[:, :, st:st + ln],
                                    xT_tiled[:, :, st:st + ln])

            # l2's W2 pool lives alongside layer 1 so its tile streaming can
            # prefetch behind W1 on the sync ring during layer 1's compute
            kxm_pool2 = octx.enter_context(tc.tile_pool(name="l2_kxm", bufs=12))

            # ---- layer 1: hT = relu(W1.T @ xT + b1), written into SBUF ----
            with ExitStack() as ctx:
                kxm_pool = ctx.enter_context(
                    tc.tile_pool(name="l1_kxm", bufs=4))

                def xT_producer(nc_, md):
                    st = seg_starts[md.n_batch_idx] + md.n_tile_idx * md.n_tile
                    return xT_sb[:, ts(md.k_tile_idx, md.k_subtiles),
                                 st:st + md.n_tile]

                def h_slices(nc_, md):
                    st = seg_starts[md.n_batch_idx] + md.n_tile_idx * md.n_tile
                    return hT[:, ts(md.m_tile_idx, md.m_subtiles),
                              st:st + md.n_tile]

                def bias_relu(nc_, psum, sbuf, md):
                    col = (md.m_tile_idx * md.m_tile) // 128 + md.m_subtile_idx
                    nc_.scalar.activation(
                        sbuf[:], psum[:],
                        mybir.ActivationFunctionType.Relu,
                        bias=b1_tile[:, col:col + 1],
                    )

                kxm_producer, kxm_shape = dma_from_dram_kxm(kxm_pool, w1.ap())
                composable_matmul_tile_kernel(
                    tc=tc,
                    kxm_shape=kxm_shape,
                    kxn_shape=ShapeInfo(pdims=((128, D // 128),), fdims=segs),
                    output_type=f16,
                    kxm_producer=kxm_producer,
                    kxn_producer=xT_producer,
                    mxn_subtile_producer=h_slices,
                    mxn_subtile_reducer=bias_relu,
                    mxn_consumer=lambda nc_, sbuf, md: None,
                    MAX_TILE_SIZE=max(segs),
                    MAX_K_TILE_SIZE=1024,
                    psum_n_bufs=2,
                )

            # ---- layer 2: oT = W2.T @ hT, hT moving so the token dim uses
            # the same exact-fit segments as layer 1 (no padding compute) ----
            with ExitStack() as ctx:
                def hT_kxn_producer(nc_, md):
                    st = seg_starts[md.n_batch_idx] + md.n_tile_idx * md.n_tile
                    return hT[:, ts(md.k_tile_idx, md.k_subtiles),
                              st:st + md.n_tile]

                oT_tiled = oT.ap().rearrange("(po pi) f -> pi po f", pi=128)

                def oT_consumer(nc_, mxn_tile, md):
                    st = seg_starts[md.n_batch_idx] + md.n_tile_idx * md.n_tile
                    nc_.sync.dma_start(
                        oT_tiled[:, ts(md.m_tile_idx, md.m_subtiles),
                                 st:st + md.n_slice_size],
                        mxn_tile[:, :, :md.n_slice_size])

                kxm_producer2, kxm_shape2 = dma_from_dram_kxm(
                    kxm_pool2, w2.ap())
                composable_matmul_tile_kernel(
                    tc=tc,
                    kxm_shape=kxm_shape2,
                    kxn_shape=ShapeInfo(pdims=((128, F // 128),), fdims=segs),
                    output_type=f32,
                    kxm_producer=kxm_producer2,
                    kxn_producer=hT_kxn_producer,
                    mxn_consumer=oT_consumer,
                    MAX_TILE_SIZE=max(segs),
                    psum_n_bufs=2,
                    temps_n_bufs=3,
                )

    nc.compile()
    _build_cache[(cap, csum)] = nc
    return nc


def _route(x2d, noise2d, Wr, br, Wn, bn):
    """Noisy top-2 router in float64. Returns (top2 ids [T,2], gates [T,2])."""
    x64 = x2d.astype(np.float64)
    logits = x64 @ Wr.astype(np.float64) + br.astype(np.float64)
    nl = x64 @ Wn.astype(np.float64) + bn.astype(np.float64)
    noisy = logits + noise2d.astype(np.float64) * np.logaddexp(0.0, nl)
    # stable argsort of -noisy == jax.lax.top_k tie-breaking (lower index wins)
    top2 = np.argsort(-noisy, axis=-1, kind="stable")[:, :TOP_K]
    v = np.take_along_axis(noisy, top2, axis=-1)
    v = v - v.max(axis=-1, keepdims=True)
    ev = np.exp(v)
    gates = ev / ev.sum(axis=-1, keepdims=True)
    return top2, gates


def kernel(x, noise, Wr, br, Wn, bn, W1, b1, W2, b2):
    x = np.ascontiguousarray(np.asarray(x, dtype=np.float32))
    x2d = x.reshape(T, D)
    top2, gates = _route(x2d, np.asarray(noise).reshape(T, E),
                         np.asarray(Wr), np.asarray(br),
                         np.asarray(Wn), np.asarray(bn))

    # dispatch: stable sort of the 2T assignments by expert id
    expert_ids = top2.ravel()  # assignment a -> expert; token = a // 2
    ord_ = np.argsort(expert_ids, kind="stable")
    counts = np.bincount(expert_ids, minlength=E)
    starts = np.zeros(E + 1, dtype=np.int64)
    np.cumsum(counts, out=starts[1:])

    cap = max(CAP, -(-int(counts.max()) // CHUNK) * CHUNK)
    csum = int(counts.max())  # layer-1 computes exactly this many columns
    nc = _build(cap, csum)

    W1 = np.asarray(W1, dtype=np.float32)
    W2 = np.asarray(W2, dtype=np.float32)
    b1 = np.asarray(b1, dtype=np.float32)
    b2 = np.asarray(b2, dtype=np.float32)
    x16 = x2d.astype(np.float16)

    in_maps = []
    for e in range(E):
        toks = ord_[starts[e]:starts[e + 1]] // 2
        xe = np.zeros((cap, D), dtype=np.float16)
        xe[:len(toks)] = x16[toks]
        in_maps.append({
            "xT": np.ascontiguousarray(xe.T),
            "w1": W1[e].astype(np.float16),
            "b1": np.ascontiguousarray(b1[e].reshape(F // 128, 128).T),
            "w2": W2[e].astype(np.float16),
        })

    res = None
    for attempt in range(3):
        try:
            res = run_bass_kernel_spmd(nc, in_maps, list(range(N_CORES)))
            break
        except Exception:
            if attempt == 2:
                raise
            import time
            time.sleep(5)
    global _last_run
    _last_run = (nc, in_maps)

    # combine: A holds expert outputs in assignment-sorted order
    A = np.empty((2 * T, D), dtype=np.float32)
    pos = np.empty(2 * T, dtype=np.int64)
    pos[ord_] = np.arange(2 * T)
    for e in range(E):
        A[starts[e]:starts[e + 1]] = res.results[e]["oT"][:, :counts[e]].T + b2[e]
    out = (gates[:, :, None] * A[pos.reshape(T, TOP_K)].astype(np.float64)).sum(axis=1)
    return out.reshape(B, S, D).astype(np.float32)

